# revision 29
# baseline (speedup 1.0000x reference)
"""GAT layer kernel for Trainium2, SPMD over 8 NeuronCores.

Reference computation (per batch b):
  h  = x @ W_lin.T                          [N, O]
  hp = concat(h, prior[None, :])            [N1, O]
  per head: hp_h = hp @ w_head[h]           [N1, O]
  t = tanh(hp_h); s_src = t @ a_src[h]; s_dst = t @ a_dst[h]
  z[i,j] = s_src[i] + s_dst[j]; y = leaky_relu(z, 0.2)
  y[mask_i | mask_j] = -1e18; p = softmax_j(y)
  out_h = p @ hp_h;  out = mean_h(out_h) + bias

Sharding: core c handles batch b=c//2 and heads h in {2*(c%2), 2*(c%2)+1}.

Key algebra: exp(leaky_relu(z)) = max(exp(z), exp(0.2 z)) and z factors as
s_src[i] + s_dst[j].  Softmax is invariant to any positive per-i scale, so
dividing by exp(0.2 s_src[i]) gives the unnormalized weights
  e[j,i] = max(r[i] * v[j], v2[j])
with r = exp(0.8 s_src'), v = exp(s_dst'), v2 = exp(0.2 s_dst'), where the
primed scores carry the mask sentinel (-400, clamped to the exp-table
range).  Each 128-row j-chunk of e is ONE fused DVE tensor_scalar
(mult, max) over bf16 operands producing bf16 weights; the PE accumulates
  avT[o, i] = sum_j V[j, o] e[j, i]   and   sums[i] = sum_j e[j, i]
flash-style in PSUM.  The host divides, fixes fully-masked rows i (whole
row masked -> reference softmax is uniform -> out row = mean_n hp_h = vbar,
exported per head), averages heads, transposes, adds bias.

Engine budget: PE streams e twice (av + sums) in bf16; score matmuls run
in float32r (TF32-like, 1 cycle/row); V / rb matmuls in bf16; all
PSUM->SBUF copies run on the otherwise-idle ACT engine; DVE does only the
fused e ops and small vector work.  Both heads' prologues are issued
before either head's j-loop so the serial score->columns chain of head 1
hides under head 0's main loop.
"""

import sys

for _p in ("/opt/trn_rl_repo",):
    if _p not in sys.path:
        sys.path.insert(0, _p)

import os as _os

import numpy as np

import concourse.bass as bass
import concourse.tile as tile
from concourse import bacc, mybir
from concourse.masks import make_identity

FP = mybir.dt.float32
FPR = mybir.dt.float32r
BF = mybir.dt.bfloat16
U8 = mybir.dt.uint8
N, N1, I, O = 2047, 2048, 256, 128
HPC = 2  # heads per core
NCORES = 8
# Mask sentinel: s' = s - 400 for masked nodes.  exp-table inputs are kept
# inside ~[-87, 88]: v2 = exp(0.2 s') >= exp(-82) needs no clamp; v and r
# inputs are clamped at CLO.  Clamped weights are ~e^-86 ~ 4e-38, vs >=
# ~e^-16 for any live entry -- negligible.
NEG = -400.0
CLO = -86.0
Tanh = mybir.ActivationFunctionType.Tanh
Exp = mybir.ActivationFunctionType.Exp
Ident = mybir.ActivationFunctionType.Identity
ALU = mybir.AluOpType

USE_FPR = not bool(_os.environ.get("GATV3_NO_FPR"))
MMDT = FPR if USE_FPR else FP
STAGE = int(_os.environ.get("GATV3_STAGE", "99"))
PE_RB = bool(_os.environ.get("GATV5_PE_RB"))
V_MM = bool(_os.environ.get("GATV6_V_MM"))


def g5(g):
    return slice(g * 512, (g + 1) * 512)


def c128(c):
    return slice(c * 128, (c + 1) * 128)


def _build() -> bass.Bass:
    nc = bacc.Bacc(None, target_bir_lowering=False, debug=False)
    x_b = nc.dram_tensor("x_b", [N1, I], BF, kind="ExternalInput")
    prior_b = nc.dram_tensor("prior_b", [O], FP, kind="ExternalInput")
    mask_b = nc.dram_tensor("mask_b", [N1], U8, kind="ExternalInput")
    W_lin = nc.dram_tensor("W_lin", [O, I], BF, kind="ExternalInput")
    w_pair = nc.dram_tensor("w_pair", [HPC, O, O], BF, kind="ExternalInput")
    a_src_p = nc.dram_tensor("a_src_p", [HPC, O], BF, kind="ExternalInput")
    a_dst_p = nc.dram_tensor("a_dst_p", [HPC, O], BF, kind="ExternalInput")
    outT = nc.dram_tensor("outT", [HPC, O, N1], FP, kind="ExternalOutput")
    sums = nc.dram_tensor("sums", [HPC, N1], FP, kind="ExternalOutput")
    vbar_out = nc.dram_tensor("vbar_out", [HPC, O], FP, kind="ExternalOutput")
    sdst_dram = nc.dram_tensor("sdst_scratch", [HPC, N1], FP)
    r_dram = nc.dram_tensor("r_scratch", [HPC, N1], BF)

    with tile.TileContext(nc) as tc:
        with (
            tc.tile_pool(name="constp", bufs=1) as constp,
            tc.tile_pool(name="bigp", bufs=1) as bigp,
            tc.tile_pool(name="headp", bufs=2) as headp,
            tc.tile_pool(name="etp", bufs=5) as etp,
            tc.tile_pool(name="outp", bufs=2) as outp,
            tc.tile_pool(name="pp", bufs=2, space="PSUM") as pp,
            tc.tile_pool(name="pav", bufs=1, space="PSUM") as pav,
            tc.tile_pool(name="psums", bufs=1, space="PSUM") as psums,
        ):
            pools = dict(constp=constp, bigp=bigp, headp=headp, etp=etp,
                         outp=outp, pp=pp, pav=pav, psums=psums, tc=tc)
            _body(nc, tc, pools,
                  x_b, prior_b, mask_b, W_lin, w_pair, a_src_p, a_dst_p,
                  outT, sums, vbar_out, sdst_dram, r_dram)
    return nc


def _body(nc, tc, pools,
          x_b, prior_b, mask_b, W_lin, w_pair, a_src_p, a_dst_p,
          outT, sums, vbar_out, sdst_dram, r_dram):
    constp, bigp, headp = pools["constp"], pools["bigp"], pools["headp"]
    etp, outp = pools["etp"], pools["outp"]
    pp, pav, psums = pools["pp"], pools["pav"], pools["psums"]
    tcx = pools["tc"]

    # ---- constants ----
    ones_row_bf = constp.tile([1, 128], BF, tag="ones_row_bf")
    nc.vector.memset(ones_row_bf, 1.0)
    ones_col_bf = constp.tile([128, 1], BF, tag="ones_col_bf")
    nc.vector.memset(ones_col_bf, 1.0)

    # mask as row (for s_src) and as 16 column chunks (for s_dst)
    m2_u8 = constp.tile([2, N1], U8, tag="m2_u8")
    nc.sync.dma_start(out=m2_u8[0:1, :], in_=mask_b[None, :])
    nc.sync.dma_start(out=m2_u8[1:2, :], in_=mask_b[None, :])
    negm2 = constp.tile([2, N1], FP, tag="negm2")
    nc.vector.tensor_scalar(negm2, m2_u8, NEG, None, op0=ALU.mult)

    # hp^T in bf16 (score and V matmuls)
    hpT_bf = bigp.tile([128, N1], BF, tag="hpT_bf")
    HW_ = {}
    with tcx.tile_pool(name="prep", bufs=1) as prep:
        # ---- W_lin / x transposed via XBAR dma transpose (bf16, no PE).
        # SP issues descriptors serially: the wlT + first x windows gate the
        # first hp matmul, so they go before everything else. ----
        wlT = prep.tile([128, 2, 128], BF, tag="wlT", bufs=1)
        for k in range(2):
            nc.sync.dma_start_transpose(out=wlT[:, k, :],
                                        in_=W_lin[:, c128(k)])
        xT0 = prep.tile([128, N1], BF, tag="xT0", bufs=1, name="xT0")
        xT1 = prep.tile([128, N1], BF, tag="xT1", bufs=1, name="xT1")
        xTk = [xT0, xT1]
        # x arrives host-padded to 2048 rows (row 2047 = zeros, the prior
        # slot) so both XBAR windows are 16-aligned with contiguous dests.
        for k in range(2):
            nc.sync.dma_start_transpose(out=xTk[k][:, 0:1024],
                                        in_=x_b[0:1024, c128(k)])

        # ---- both heads' weights (small DMAs, off the critical path) ----
        for h in range(HPC):
            wh_bf = headp.tile([128, 128], BF, tag="wh_bf")
            nc.sync.dma_start(out=wh_bf, in_=w_pair[h])
            acols = headp.tile([128, 2], BF, tag="acols")
            nc.sync.dma_start(out=acols[:, 0:1], in_=a_src_p[h][:, None])
            nc.sync.dma_start(out=acols[:, 1:2], in_=a_dst_p[h][:, None])
            HW_[h] = dict(wh_bf=wh_bf, acols=acols)

        for k in range(2):
            nc.sync.dma_start_transpose(out=xTk[k][:, 1024:2048],
                                        in_=x_b[1024:2048, c128(k)])
        prior_sb = prep.tile([128, 1], FP, tag="prior_sb", bufs=1)
        nc.sync.dma_start(out=prior_sb, in_=prior_b[:, None])

        # ---- hpT[o, n] = (x @ W_lin.T).T in bf16, col 2047 = prior ----
        for g in range(4):
            ph = pp.tile([128, 512], FP, tag="tr")
            for k in range(2):
                nc.tensor.matmul(ph, wlT[:, k, :], xTk[k][:, g5(g)],
                                 start=(k == 0), stop=(k == 1))
            nc.scalar.activation(hpT_bf[:, g5(g)], ph, Ident)
        nc.scalar.activation(hpT_bf[:, 2047:2048], prior_sb, Ident)

    # column sums of hp (for vbar = (hpbar @ w_head) / N1)
    hpbar_col = constp.tile([128, 1], FP, tag="hpbar_col")
    nc.vector.reduce_sum(hpbar_col, hpT_bf, axis=mybir.AxisListType.X)
    hpbar_bf = constp.tile([128, 1], BF, tag="hpbar_bf")
    nc.vector.tensor_copy(hpbar_bf, hpbar_col)

    if STAGE == 1:
        pass
        return

    # ================= phase A1: scores / V / bounce =================
    H = {}
    for h in range(HPC):
        wh_bf = HW_[h]["wh_bf"]
        acols = HW_[h]["acols"]

        # tanh(hp_h^T)
        tT = headp.tile([128, N1], BF, tag="tT")
        for g in range(4):
            php = pp.tile([128, 512], FP, tag="tr")
            nc.tensor.matmul(php, wh_bf, hpT_bf[:, g5(g)], start=True, stop=True)
            nc.scalar.activation(tT[:, g5(g)], php, Tanh)

        # s' rows: [a_src | a_dst]^T @ tT + NEG*mask
        s2row = headp.tile([2, N1], FP, tag="s2row")
        for g in range(4):
            ps1 = pp.tile([128, 512], FP, tag="tr")
            nc.tensor.matmul(ps1[:2, :], acols, tT[:, g5(g)],
                             start=True, stop=True)
            nc.vector.tensor_tensor(s2row[:, g5(g)], ps1[:2, :],
                                    negm2[:, g5(g)], op=ALU.add)
        # s_dst' to column layout via DRAM bounce (latency hidden: consumed
        # only at this head's j-loop)
        nc.sync.dma_start(out=sdst_dram[h, :], in_=s2row[1:2, :])
        sdc = headp.tile([128, 16], FP, tag="sdc")
        nc.sync.dma_start(out=sdc,
                          in_=sdst_dram[h, :].rearrange("(c p) -> p c", p=128))

        # V = hp_h natural [n, p] (bf16)
        V = headp.tile([128, N1], BF, tag="V")
        for t in range(16):
            pv = pp.tile([128, 512], FP, tag="tr")
            nc.tensor.matmul(pv[:, :128], hpT_bf[:, c128(t)], wh_bf,
                             start=True, stop=True)
            nc.scalar.activation(V[:, c128(t)], pv[:, :128], Ident)
        pvb = pp.tile([128, 512], FP, tag="tr")
        nc.tensor.matmul(pvb[:1, :128], hpbar_bf, wh_bf, start=True, stop=True)
        vbar_sb = headp.tile([1, 128], FP, tag="vbar_sb")
        nc.vector.tensor_scalar_mul(vbar_sb, pvb[:1, :128], 1.0 / N1)
        nc.sync.dma_start(out=vbar_out[h, :], in_=vbar_sb)

        H[h] = dict(V=V, s2row=s2row, sdc=sdc)

    # ================= phase A2: r broadcast + column exps =================
    for h in range(HPC):
        s2row, sdc = H[h]["s2row"], H[h]["sdc"]
        sdc_c = headp.tile([128, 16], FP, tag="sdc_c")
        nc.vector.tensor_scalar_max(sdc_c, sdc, CLO)
        v_col = headp.tile([128, 16], FP, tag="v_col")
        nc.scalar.activation(v_col, sdc_c, Exp)
        # 0.2 * sdc >= -82: already inside the exp table range
        v2_col = headp.tile([128, 16], FP, tag="v2_col")
        nc.scalar.activation(v2_col, sdc, Exp, scale=0.2)

        rr_tmp = headp.tile([1, N1], FP, tag="rr_tmp")
        nc.vector.tensor_scalar(rr_tmp, s2row[0:1, :], 0.8, CLO,
                                op0=ALU.mult, op1=ALU.max)
        r_row = headp.tile([1, N1], BF, tag="r_row")
        nc.scalar.activation(r_row, rr_tmp, Exp)
        rb = headp.tile([128, N1], BF, tag="rb")
        if PE_RB:
            for g in range(4):
                prb = pp.tile([128, 512], FP, tag="tr")
                nc.tensor.matmul(prb, ones_row_bf, r_row[:, g5(g)],
                                 start=True, stop=True)
                nc.scalar.activation(rb[:, g5(g)], prb, Ident)
        else:
            nc.sync.dma_start(out=r_dram[h, :], in_=r_row)
            nc.sync.dma_start(
                out=rb, in_=r_dram[h][None, :].to_broadcast((128, N1)))
        H[h].update(rb=rb, v_col=v_col, v2_col=v2_col)

    if STAGE == 2:
        return

    # ================= phase B: per-head j-loops =================
    for h in range(HPC):
        V, rb = H[h]["V"], H[h]["rb"]
        v_col, v2_col = H[h]["v_col"], H[h]["v2_col"]

        av = pav.tile([128, N1], FP, tag="av")
        # 4 per-i-group row-sum accumulators, packed two per PSUM bank at
        # the legal matmul output partition bases (0 and 32).
        sumpA = psums.tile([33, 512], FP, tag="sumpA")
        sumpB = psums.tile([33, 512], FP, tag="sumpB")

        def sum_slot(g):
            t = sumpA if g < 2 else sumpB
            base = 32 * (g % 2)
            return t[base:base + 1, :]

        for jc in range(16):
            eT = etp.tile([128, N1], BF, tag="eT")
            nc.vector.tensor_scalar(eT, rb, v_col[:, jc:jc + 1],
                                    v2_col[:, jc:jc + 1],
                                    op0=ALU.mult, op1=ALU.max)
            for g in range(4):
                nc.tensor.matmul(av[:, g5(g)], V[:, c128(jc)], eT[:, g5(g)],
                                 start=(jc == 0), stop=(jc == 15),
                                 skip_group_check=True)
            for g in range(4):
                nc.tensor.matmul(sum_slot(g), ones_col_bf, eT[:, g5(g)],
                                 start=(jc == 0), stop=(jc == 15),
                                 skip_group_check=True)

        # ---- export unnormalized av and the sums; host divides ----
        sum_sb = headp.tile([1, N1], FP, tag="sum_sb")
        for g in range(4):
            if g < 2:
                nc.scalar.activation(sum_sb[:, g5(g)], sum_slot(g), Ident)
            else:
                nc.vector.tensor_copy(sum_sb[:, g5(g)], sum_slot(g))
        nc.sync.dma_start(out=sums[h, :], in_=sum_sb)
        for g in range(4):
            outF = outp.tile([128, 512], FP, tag="outF")
            if g % 2 == 0:
                nc.scalar.activation(outF, av[:, g5(g)], Ident)
            else:
                nc.vector.tensor_copy(outF, av[:, g5(g)])
            nc.sync.dma_start(out=outT[h, :, g5(g)], in_=outF)


_NC_CACHE = None


def _get_nc():
    global _NC_CACHE
    if _NC_CACHE is None:
        nc = _build()
        nc.finalize()
        _NC_CACHE = nc
    return _NC_CACHE


def make_in_maps(x, prior_feature, x_mask, W_lin, w_head, a_src, a_dst):
    bf16 = np.dtype(mybir.dt.np(BF))
    x = np.asarray(x, np.float32).astype(bf16)
    x = np.ascontiguousarray(np.concatenate(
        [x, np.zeros((x.shape[0], 1, I), bf16)], axis=1))
    prior_feature = np.ascontiguousarray(np.asarray(prior_feature, np.float32))
    x_mask_u8 = np.ascontiguousarray(np.asarray(x_mask).astype(np.uint8))
    W_lin = np.ascontiguousarray(np.asarray(W_lin, np.float32).astype(bf16))
    w_head = np.ascontiguousarray(np.asarray(w_head, np.float32).astype(bf16))
    a_src = np.ascontiguousarray(np.asarray(a_src, np.float32).astype(bf16))
    a_dst = np.ascontiguousarray(np.asarray(a_dst, np.float32).astype(bf16))
    in_maps = []
    for c in range(NCORES):
        b, h0 = c // 2, (c % 2) * HPC
        in_maps.append(dict(
            x_b=x[b],
            prior_b=prior_feature[b],
            mask_b=x_mask_u8[b],
            W_lin=W_lin,
            w_pair=np.ascontiguousarray(w_head[h0:h0 + HPC]),
            a_src_p=np.ascontiguousarray(a_src[h0:h0 + HPC]),
            a_dst_p=np.ascontiguousarray(a_dst[h0:h0 + HPC]),
        ))
    return in_maps


def combine_results(results, x_mask, bias):
    x_mask = np.asarray(x_mask).astype(bool)
    out = np.zeros((4, N1, O), np.float32)
    for c in range(NCORES):
        b = c // 2
        o = results[c]["outT"]       # [HPC, O, N1] unnormalized
        s = results[c]["sums"]       # [HPC, N1] softmax denominators
        vb = results[c]["vbar_out"]  # [HPC, O] masked-row fill value
        m = x_mask[b]
        acc = np.zeros((O, N1), np.float32)
        for k in range(HPC):
            oh = o[k] / s[k][None, :]
            oh[:, m] = vb[k][:, None]
            acc += oh
        out[b] += acc.T * 0.25
    out += np.asarray(bias, np.float32)[None, None, :]
    return out


def kernel(x, prior_feature, x_mask, W_lin, w_head, a_src, a_dst, bias,
           **run_kwargs):
    from concourse.bass_utils import run_bass_kernel_spmd
    nc = _get_nc()
    in_maps = make_in_maps(x, prior_feature, x_mask, W_lin, w_head,
                           a_src, a_dst)
    br = run_bass_kernel_spmd(nc, in_maps, core_ids=list(range(NCORES)),
                              **run_kwargs)
    out = combine_results(br.results, x_mask, bias)
    if run_kwargs:
        kernel.last_bass_results = br
    return out
